# revision 4
# baseline (speedup 1.0000x reference)
"""8-core Trainium2 Bass kernel for nn_Attention_89489938579587.

reference: qkv = x @ w_attn.T; split q,k,v per 16 heads (HD=128); RoPE
(interleaved pairs); non-causal SDPA; y @ w_proj.T.  B=4, T=2048, D=2048.

Sharding: core i -> batch b=i//2, query-token half i%2 (1024 tokens).
Each core computes QKV for its batch (K,V over all 2048 kv tokens, Q over
its 1024), RoPE, SDPA for all 16 heads over its queries, and the output
projection rows for its tokens.  Host concatenates 8 x [1024, 2048].
No collectives.  Matmuls run in float32r (full PE rate, ~1e-4 rel err).

Self-contained: builds the Bass program on first call, runs via
run_bass_kernel_spmd on cores 0-7.
"""

import numpy as np
from contextlib import ExitStack

import concourse.bass as bass
import concourse.tile as tile
from concourse import mybir
from concourse.bass import ts

# ---------------------------------------------------------------------------
# Workarounds for this toolchain:
# 1) walrus here rejects any instruction with >1 semaphore wait ("Too many
#    sync wait commands").  After Tile lowering, split extra waits onto
#    same-engine InstNoOp instructions inserted right before the offender.
# 2) the Tile tail drain carries many waits; patch _drain_and_barrier to put
#    them on SP nops (one each) before a waitless drain.
# ---------------------------------------------------------------------------
import bass_rust


def _split_multi_waits(nc, max_waits=1):
    n = 0
    for fn in nc.m.functions:
        for blk in fn.blocks:
            insts = blk.instructions
            i = 0
            while i < len(insts):
                inst = insts[i]
                si = inst.sync_info
                waits = list(si.on_wait) if (si is not None and si.on_wait) else []
                if len(waits) > max_waits:
                    si.on_wait = waits[:max_waits]
                    extra = waits[max_waits:]
                    for j in range(0, len(extra), max_waits):
                        nop = mybir.InstNoOp(
                            name=nc.get_next_instruction_name(), ins=[], outs=[])
                        nop.engine = inst.engine
                        nop.sync_info = bass_rust.SyncInfo(
                            on_wait=extra[j:j + max_waits], on_update=[])
                        nc.register_instruction(nop, overwrite=True)
                        insts.insert(i, nop)
                        i += 1
                        n += 1
                i += 1
    return n


def _patched_drain_and_barrier(self, tick_clock, wait_clock):
    from concourse.vector_clock import ScopedClock
    nc = self.nc
    probe = nc.sync.nop()
    wait_clock.add_sem_waits(probe.ins, ScopedClock({None: tick_clock.global_clock}))
    si = probe.ins.sync_info
    waits = list(si.on_wait or []) if si is not None else []
    if len(waits) > 1:
        si.on_wait = [waits[0]]
        for w in waits[1:]:
            nop = nc.sync.nop()
            nsi = nop.ins.sync_info
            if nsi is None:
                nop.ins.sync_info = bass_rust.SyncInfo(on_wait=[w], on_update=[])
            else:
                nsi.on_wait = [w]
    nc.sync.drain()
    nc.all_engine_barrier()
    assert self.sems is not None
    popped = nc._tile_sem_poison_stack.pop()
    assert popped is self._sem_poison
    nc.clear_and_free_semaphores(list(self.sems.allocated().values()))
    nc.all_engine_barrier()


_patched = False


def _apply_patches():
    global _patched
    if not _patched:
        tile.TileContext._drain_and_barrier = _patched_drain_and_barrier
        _patched = True


# ---------------------------------------------------------------------------
# Problem constants (hardcoded per spec)
# ---------------------------------------------------------------------------
F32R = mybir.dt.float32r
F32 = mybir.dt.float32
EXP = mybir.ActivationFunctionType.Exp

B, T, D, H, HD = 4, 2048, 2048, 16, 128
CC = D // 128           # contraction chunks
NTQ = 1024              # query tokens per core
NTKV = T                # kv tokens per core
KC = NTKV // 128        # kv chunks
SCALE = 1.0 / float(np.sqrt(HD))
N_CORES = 8


def build_nc(n_cores=N_CORES):
    _apply_patches()
    nc = bass.Bass("TRN2", target_bir_lowering=False, debug=False,
                   num_devices=n_cores)
    xT = nc.dram_tensor("xT", [D, T], F32R, kind="ExternalInput").ap()
    xTq = nc.dram_tensor("xTq", [D, NTQ], F32R, kind="ExternalInput").ap()
    wqT = nc.dram_tensor("wqT", [D, D], F32R, kind="ExternalInput").ap()
    wkT = nc.dram_tensor("wkT", [D, D], F32R, kind="ExternalInput").ap()
    wvT = nc.dram_tensor("wvT", [D, D], F32R, kind="ExternalInput").ap()
    wpT = nc.dram_tensor("wpT", [D, D], F32R, kind="ExternalInput").ap()
    cs2q = nc.dram_tensor("cs2q", [128, NTQ], F32, kind="ExternalInput").ap()
    sn2q = nc.dram_tensor("sn2q", [128, NTQ], F32, kind="ExternalInput").ap()
    cs2k = nc.dram_tensor("cs2k", [128, NTKV], F32, kind="ExternalInput").ap()
    sn2k = nc.dram_tensor("sn2k", [128, NTKV], F32, kind="ExternalInput").ap()
    onesd = nc.dram_tensor("onesd", [128, 128], F32R, kind="ExternalInput").ap()
    out = nc.dram_tensor("out", [NTQ, D], F32, kind="ExternalOutput").ap()

    qTs = nc.dram_tensor("qTs", [D, NTQ], F32R).ap()
    kTs = nc.dram_tensor("kTs", [D, NTKV], F32R).ap()
    vs = nc.dram_tensor("vs", [NTKV, D], F32R).ap()
    oTs = nc.dram_tensor("oTs", [D, NTQ], F32R).ap()

    xT_r = xT.rearrange("(cc p) t -> cc p t", p=128)
    xTq_r = xTq.rearrange("(cc p) t -> cc p t", p=128)
    wqT_r = wqT.rearrange("(cc p) f -> cc p f", p=128)
    wkT_r = wkT.rearrange("(cc p) f -> cc p f", p=128)
    wvT_r = wvT.rearrange("(cc p) f -> cc p f", p=128)
    wpT_r = wpT.rearrange("(cc p) f -> cc p f", p=128)
    vs_r = vs.rearrange("(tc p) f -> tc p f", p=128)
    oTs_r = oTs.rearrange("(hc p) t -> p hc t", p=128)

    with tile.TileContext(nc) as tc, ExitStack() as octx:
        cs_pool = octx.enter_context(tc.tile_pool(name="cs", bufs=1))
        csq_sb = cs_pool.tile([128, NTQ], F32, tag="csq")
        snq_sb = cs_pool.tile([128, NTQ], F32, tag="snq")
        csk_sb = cs_pool.tile([128, NTKV], F32, tag="csk")
        snk_sb = cs_pool.tile([128, NTKV], F32, tag="snk")
        nc.sync.dma_start(csq_sb[:], cs2q[:])
        nc.sync.dma_start(snq_sb[:], sn2q[:])
        nc.sync.dma_start(csk_sb[:], cs2k[:])
        nc.sync.dma_start(snk_sb[:], sn2k[:])

        # ---------------- phase 1: QKV projections + RoPE ----------------
        with ExitStack() as p1:
            xt_pool = p1.enter_context(tc.tile_pool(name="xt", bufs=18))
            wqk_pool = p1.enter_context(tc.tile_pool(name="wqk", bufs=34))
            wv_pool = p1.enter_context(tc.tile_pool(name="wv", bufs=18))
            ev_pool = p1.enter_context(tc.tile_pool(name="ev", bufs=4))
            rp_pool = p1.enter_context(tc.tile_pool(name="rp", bufs=3))
            ps1 = p1.enter_context(tc.tile_pool(name="ps1", bufs=4, space="PSUM"))

            def rope_evict(ps, cs_sb, sn_sb, toff, dst_r, h):
                sf = rp_pool.tile([128, 512], F32, tag="sf")
                nc.scalar.copy(sf[:], ps[:])
                sw = rp_pool.tile([128, 512], F32, tag="sw")
                nc.sync.dma_start(sw[0:64, :], sf[64:128, :])
                nc.sync.dma_start(sw[64:128, :], sf[0:64, :])
                ta = rp_pool.tile([128, 512], F32, tag="ta")
                nc.vector.tensor_mul(ta[:], sf[:], cs_sb[:, toff:toff + 512])
                tb = rp_pool.tile([128, 512], F32, tag="tb")
                nc.vector.tensor_mul(tb[:], sw[:], sn_sb[:, toff:toff + 512])
                o = ev_pool.tile([128, 512], F32R, tag="ev")
                nc.vector.tensor_add(o[:], ta[:], tb[:])
                nc.sync.dma_start(dst_r[h * 128:(h + 1) * 128, toff:toff + 512], o[:])

            # --- Q projection (local tokens) ---
            xq = []
            for cc in range(CC):
                t_ = xt_pool.tile([128, 1024], F32R, tag="xt")
                nc.sync.dma_start(t_[:], xTq_r[cc])
                xq.append(t_)
            for h in range(H):
                wsl = []
                for cc in range(CC):
                    w_ = wqk_pool.tile([128, 128], F32R, tag="wqk")
                    nc.sync.dma_start(w_[:], wqT_r[cc, :, h * 128:(h + 1) * 128])
                    wsl.append(w_)
                for tt in range(NTQ // 512):
                    ps = ps1.tile([128, 512], F32, tag="ps1")
                    for cc in range(CC):
                        nc.tensor.matmul(ps[:], wsl[cc][:], xq[cc][:, ts(tt, 512)],
                                         start=(cc == 0), stop=(cc == CC - 1))
                    rope_evict(ps, csq_sb, snq_sb, tt * 512, qTs, h)

            # --- K^T and V per kv t-half ---
            for th in range(2):
                xk = []
                for cc in range(CC):
                    t_ = xt_pool.tile([128, 1024], F32R, tag="xt")
                    nc.sync.dma_start(t_[:], xT_r[cc, :, th * 1024:(th + 1) * 1024])
                    xk.append(t_)
                for h in range(H):
                    wsl = []
                    for cc in range(CC):
                        w_ = wqk_pool.tile([128, 128], F32R, tag="wqk")
                        nc.sync.dma_start(w_[:], wkT_r[cc, :, h * 128:(h + 1) * 128])
                        wsl.append(w_)
                    for tt in range(2):
                        ps = ps1.tile([128, 512], F32, tag="ps1")
                        for cc in range(CC):
                            nc.tensor.matmul(ps[:], wsl[cc][:], xk[cc][:, ts(tt, 512)],
                                             start=(cc == 0), stop=(cc == CC - 1))
                        rope_evict(ps, csk_sb, snk_sb, th * 1024 + tt * 512, kTs, h)
                for ft in range(4):
                    wvl = []
                    for cc in range(CC):
                        w_ = wv_pool.tile([128, 512], F32R, tag="wv")
                        nc.sync.dma_start(w_[:], wvT_r[cc, :, ft * 512:(ft + 1) * 512])
                        wvl.append(w_)
                    for tch in range(8):
                        ps = ps1.tile([128, 512], F32, tag="ps1")
                        for cc in range(CC):
                            nc.tensor.matmul(ps[:], xk[cc][:, ts(tch, 128)], wvl[cc][:],
                                             start=(cc == 0), stop=(cc == CC - 1))
                        o = ev_pool.tile([128, 512], F32R, tag="ev")
                        nc.scalar.copy(o[:], ps[:])
                        nc.sync.dma_start(
                            vs_r[th * 8 + tch, :, ft * 512:(ft + 1) * 512], o[:])

        # ---------------- phase 2: SDPA per head ----------------
        with ExitStack() as p2:
            qh_pool = p2.enter_context(tc.tile_pool(name="qh", bufs=2))
            kh_pool = p2.enter_context(tc.tile_pool(name="kh", bufs=2))
            vh_pool = p2.enter_context(tc.tile_pool(name="vh", bufs=2))
            e_pool = p2.enter_context(tc.tile_pool(name="eT", bufs=4))
            es_pool = p2.enter_context(tc.tile_pool(name="es", bufs=3))
            oev_pool = p2.enter_context(tc.tile_pool(name="oev", bufs=3))
            one_pool = p2.enter_context(tc.tile_pool(name="one", bufs=1))
            rs_pool = p2.enter_context(tc.tile_pool(name="rs", bufs=3))
            s_ps_pool = p2.enter_context(tc.tile_pool(name="sps", bufs=3, space="PSUM"))
            o_ps_pool = p2.enter_context(tc.tile_pool(name="ops", bufs=2, space="PSUM"))
            m_ps_pool = p2.enter_context(tc.tile_pool(name="mps", bufs=2, space="PSUM"))

            ones128 = one_pool.tile([128, 1], F32R, tag="o128")
            nc.sync.dma_start(ones128[:], onesd[:, 0:1])
            ones1 = one_pool.tile([1, 128], F32R, tag="o1")
            nc.sync.dma_start(ones1[:], onesd[0:1, :])

            for h in range(H):
                qh_sb = qh_pool.tile([128, NTQ], F32R, tag="qh")
                nc.sync.dma_start(qh_sb[:], qTs[h * 128:(h + 1) * 128, :])
                kh_sb = kh_pool.tile([128, NTKV], F32R, tag="kh")
                nc.sync.dma_start(kh_sb[:], kTs[h * 128:(h + 1) * 128, :])
                vh_sb = vh_pool.tile([128, KC, 128], F32R, tag="vh")
                nc.sync.dma_start(
                    vh_sb[:],
                    vs.rearrange("(tc p) f -> p tc f", p=128)[:, :, h * 128:(h + 1) * 128])
                for qh in range(NTQ // 512):
                    qsl = qh_sb[:, ts(qh, 512)]
                    esum = es_pool.tile([128, 512], F32R, tag="es")
                    o_ps = o_ps_pool.tile([128, 512], F32, tag="ops")
                    for kc in range(KC):
                        s_ps = s_ps_pool.tile([128, 512], F32, tag="sps")
                        nc.tensor.matmul(s_ps[:], kh_sb[:, ts(kc, 128)], qsl,
                                         start=True, stop=True)
                        eT = e_pool.tile([128, 512], F32R, tag="eT")
                        nc.scalar.activation(eT[:], s_ps[:], EXP, scale=SCALE)
                        if kc == 0:
                            nc.vector.tensor_copy(esum[:], eT[:])
                        else:
                            nc.vector.tensor_add(esum[:], esum[:], eT[:])
                        nc.tensor.matmul(o_ps[:], vh_sb[:, kc, :], eT[:],
                                         start=(kc == 0), stop=(kc == KC - 1))
                    m_ps = m_ps_pool.tile([128, 512], F32, tag="mps")
                    nc.tensor.matmul(m_ps[0:1, :], ones128[:], esum[:],
                                     start=True, stop=True)
                    rs = rs_pool.tile([1, 512], F32R, tag="rs")
                    with nc.allow_low_precision(reason="f32r is 4-byte"):
                        nc.vector.reciprocal(rs[:], m_ps[0:1, :])
                    nc.tensor.matmul(m_ps[:, :], ones1[:], rs[:],
                                     start=True, stop=True)
                    bc_sb = rs_pool.tile([128, 512], F32, tag="bc")
                    nc.scalar.copy(bc_sb[:], m_ps[:])
                    oT_ev = oev_pool.tile([128, 512], F32R, tag="oev")
                    nc.vector.tensor_mul(oT_ev[:], o_ps[:], bc_sb[:])
                    nc.sync.dma_start(
                        oTs[h * 128:(h + 1) * 128, ts(qh, 512)], oT_ev[:])

        # ---------------- phase 3: output projection ----------------
        with ExitStack() as p3:
            wp_pool = p3.enter_context(tc.tile_pool(name="wp", bufs=18))
            o3_pool = p3.enter_context(tc.tile_pool(name="o3", bufs=2))
            outev_pool = p3.enter_context(tc.tile_pool(name="outev", bufs=4))
            ps3 = p3.enter_context(tc.tile_pool(name="ps3", bufs=4, space="PSUM"))

            for ft in range(4):
                wpl = []
                for hc in range(CC):
                    w_ = wp_pool.tile([128, 512], F32R, tag="wp")
                    nc.sync.dma_start(w_[:], wpT_r[hc, :, ft * 512:(ft + 1) * 512])
                    wpl.append(w_)
                for tch in range(NTQ // 128):
                    osl = o3_pool.tile([128, H, 128], F32R, tag="o3")
                    nc.sync.dma_start(osl[:], oTs_r[:, :, ts(tch, 128)])
                    ps = ps3.tile([128, 512], F32, tag="ps3")
                    for hc in range(H):
                        nc.tensor.matmul(ps[:], osl[:, hc, :], wpl[hc][:],
                                         start=(hc == 0), stop=(hc == H - 1))
                    oev = outev_pool.tile([128, 512], F32, tag="outev")
                    nc.scalar.copy(oev[:], ps[:])
                    nc.sync.dma_start(
                        out[ts(tch, 128), ft * 512:(ft + 1) * 512], oev[:])

    _split_multi_waits(nc)
    return nc


# ---------------------------------------------------------------------------
# host-side prep / assembly
# ---------------------------------------------------------------------------

_ONES = np.ones((128, 128), dtype=np.float32)


def prep_inputs(x, w_attn, w_proj):
    x = np.asarray(x, dtype=np.float32)
    w_attn = np.asarray(w_attn, dtype=np.float32)
    w_proj = np.asarray(w_proj, dtype=np.float32)

    perm = np.concatenate([np.arange(0, HD, 2), np.arange(1, HD, 2)])
    colperm = (np.arange(H)[:, None] * HD + perm[None, :]).ravel()

    wq, wk, wv = w_attn[0:D], w_attn[D:2 * D], w_attn[2 * D:3 * D]
    wqT_p = np.ascontiguousarray(wq.T[:, colperm])
    wkT_p = np.ascontiguousarray(wk.T[:, colperm])
    wvT = np.ascontiguousarray(wv.T)
    wpT = np.ascontiguousarray(w_proj.T)

    inv = 1.0 / (10000.0 ** (np.arange(0, HD, 2, dtype=np.float64) / HD))
    fr = np.outer(np.arange(T, dtype=np.float64), inv)
    cos = np.cos(fr).T
    sin = np.sin(fr).T
    cs2 = np.concatenate([cos, cos], 0).astype(np.float32)
    sn2 = np.concatenate([-sin, sin], 0).astype(np.float32)

    in_maps = []
    for i in range(N_CORES):
        b, half = i // 2, i % 2
        q0 = half * NTQ
        xT_b = np.ascontiguousarray(x[b].T)
        in_maps.append({
            "xT": xT_b,
            "xTq": np.ascontiguousarray(xT_b[:, q0:q0 + NTQ]),
            "wqT": wqT_p, "wkT": wkT_p, "wvT": wvT, "wpT": wpT,
            "cs2q": np.ascontiguousarray(cs2[:, q0:q0 + NTQ]),
            "sn2q": np.ascontiguousarray(sn2[:, q0:q0 + NTQ]),
            "cs2k": cs2, "sn2k": sn2,
            "onesd": _ONES,
        })
    return in_maps


def assemble(results):
    out = np.empty((B, T, D), dtype=np.float32)
    for i in range(N_CORES):
        b, half = i // 2, i % 2
        out[b, half * NTQ:(half + 1) * NTQ, :] = results[i]["out"]
    return out


_nc_cache = None


def _get_nc():
    global _nc_cache
    if _nc_cache is None:
        _nc_cache = build_nc()
    return _nc_cache


def kernel(x, w_attn, w_proj):
    from concourse.bass_utils import run_bass_kernel_spmd
    nc = _get_nc()
    in_maps = prep_inputs(x, w_attn, w_proj)
    res = run_bass_kernel_spmd(nc, in_maps, list(range(N_CORES)))
    return assemble(res.results)


def run_profiled(x, w_attn, w_proj, trace_cores=None):
    """Like kernel() but with NTFF profiling; returns BassKernelResults."""
    from concourse.bass_utils import run_bass_kernel_spmd
    import sys as _sys, types as _types, contextlib as _cl
    # inject the missing antenv.axon_hooks so trace=True works on this image
    try:
        import antenv
        if "antenv.axon_hooks" not in _sys.modules:
            mod = _types.ModuleType("antenv.axon_hooks")
            _h = [None]
            mod.set_axon_ntff_profile_hook = lambda h: _h.__setitem__(0, h)
            mod.get_axon_ntff_profile_hook = lambda: _h[0]
            _sys.modules["antenv.axon_hooks"] = mod
            antenv.axon_hooks = mod
            from trn_agent_boot.trn_boot import _ntff_profile_via_ctypes
            mod.set_axon_ntff_profile_hook(
                _ntff_profile_via_ctypes('/opt/axon/libaxon_pjrt.so'))
    except Exception as e:  # profiling is best-effort
        print("profile hook setup failed:", e)
    nc = _get_nc()
    in_maps = prep_inputs(x, w_attn, w_proj)
    return run_bass_kernel_spmd(
        nc, in_maps, list(range(N_CORES)), trace=True,
        trace_cores=trace_cores if trace_cores is not None else [0])


# revision 5
# speedup vs baseline: 1.1675x; 1.1675x over previous
"""8-core Trainium2 Bass kernel for nn_Attention_89489938579587.

reference: qkv = x @ w_attn.T; split q,k,v per 16 heads (HD=128); RoPE
(interleaved pairs); non-causal SDPA; y @ w_proj.T.  B=4, T=2048, D=2048.

Sharding: core i -> batch b=i//2, query-token half i%2 (1024 tokens).
Each core computes QKV for its batch (K,V over all 2048 kv tokens, Q over
its 1024), RoPE, SDPA for all 16 heads over its queries, and the output
projection rows for its tokens.  Host concatenates 8 x [1024, 2048].
No collectives.  Matmuls run in float32r (full PE rate, ~1e-4 rel err).

Weights and DRAM scratch are stored in matmul-tile-blocked layouts so every
DMA moves large contiguous DRAM segments (descriptor-count, not byte-count,
was the V0 DMA bottleneck).  Attention outputs stay resident in SBUF.

Self-contained: builds the Bass program on first call, runs via
run_bass_kernel_spmd on cores 0-7.
"""

import numpy as np
from contextlib import ExitStack

import concourse.bass as bass
import concourse.tile as tile
from concourse import mybir
from concourse.bass import ts

# ---------------------------------------------------------------------------
# Workarounds for this toolchain:
# 1) walrus here rejects any instruction with >1 semaphore wait ("Too many
#    sync wait commands").  After Tile lowering, split extra waits onto
#    same-engine InstNoOp instructions inserted right before the offender.
# 2) the Tile tail drain carries many waits; patch _drain_and_barrier to put
#    them on SP nops (one each) before a waitless drain.
# ---------------------------------------------------------------------------
import bass_rust


def _split_multi_waits(nc, max_waits=1):
    n = 0
    for fn in nc.m.functions:
        for blk in fn.blocks:
            insts = blk.instructions
            i = 0
            while i < len(insts):
                inst = insts[i]
                si = inst.sync_info
                waits = list(si.on_wait) if (si is not None and si.on_wait) else []
                if len(waits) > max_waits:
                    si.on_wait = waits[:max_waits]
                    extra = waits[max_waits:]
                    for j in range(0, len(extra), max_waits):
                        nop = mybir.InstNoOp(
                            name=nc.get_next_instruction_name(), ins=[], outs=[])
                        nop.engine = inst.engine
                        nop.sync_info = bass_rust.SyncInfo(
                            on_wait=extra[j:j + max_waits], on_update=[])
                        nc.register_instruction(nop, overwrite=True)
                        insts.insert(i, nop)
                        i += 1
                        n += 1
                i += 1
    return n


def _patched_drain_and_barrier(self, tick_clock, wait_clock):
    from concourse.vector_clock import ScopedClock
    nc = self.nc
    probe = nc.sync.nop()
    wait_clock.add_sem_waits(probe.ins, ScopedClock({None: tick_clock.global_clock}))
    si = probe.ins.sync_info
    waits = list(si.on_wait or []) if si is not None else []
    if len(waits) > 1:
        si.on_wait = [waits[0]]
        for w in waits[1:]:
            nop = nc.sync.nop()
            nsi = nop.ins.sync_info
            if nsi is None:
                nop.ins.sync_info = bass_rust.SyncInfo(on_wait=[w], on_update=[])
            else:
                nsi.on_wait = [w]
    nc.sync.drain()
    nc.all_engine_barrier()
    assert self.sems is not None
    popped = nc._tile_sem_poison_stack.pop()
    assert popped is self._sem_poison
    nc.clear_and_free_semaphores(list(self.sems.allocated().values()))
    nc.all_engine_barrier()


_patched = False


def _apply_patches():
    global _patched
    if not _patched:
        tile.TileContext._drain_and_barrier = _patched_drain_and_barrier
        _patched = True


# ---------------------------------------------------------------------------
# Problem constants (hardcoded per spec)
# ---------------------------------------------------------------------------
F32R = mybir.dt.float32r
F32 = mybir.dt.float32
EXP = mybir.ActivationFunctionType.Exp

B, T, D, H, HD = 4, 2048, 2048, 16, 128
CC = D // 128           # contraction chunks
NTQ = 1024              # query tokens per core
NTKV = T                # kv tokens per core
KC = NTKV // 128        # kv chunks
SCALE = 1.0 / float(np.sqrt(HD))
N_CORES = 8


def build_nc(n_cores=N_CORES):
    _apply_patches()
    nc = bass.Bass("TRN2", target_bir_lowering=False, debug=False,
                   num_devices=n_cores)
    xT = nc.dram_tensor("xT", [D, T], F32R, kind="ExternalInput").ap()
    xTq = nc.dram_tensor("xTq", [D, NTQ], F32R, kind="ExternalInput").ap()
    # blocked weights: [h, cc, 128, 128] / [ft, cc, 128, 512]
    wqb = nc.dram_tensor("wqb", [H, CC, 128, 128], F32R, kind="ExternalInput").ap()
    wkb = nc.dram_tensor("wkb", [H, CC, 128, 128], F32R, kind="ExternalInput").ap()
    wvb = nc.dram_tensor("wvb", [4, CC, 128, 512], F32R, kind="ExternalInput").ap()
    wpb = nc.dram_tensor("wpb", [4, CC, 128, 512], F32R, kind="ExternalInput").ap()
    cs2q = nc.dram_tensor("cs2q", [128, NTQ], F32, kind="ExternalInput").ap()
    sn2q = nc.dram_tensor("sn2q", [128, NTQ], F32, kind="ExternalInput").ap()
    cs2k = nc.dram_tensor("cs2k", [128, NTKV], F32, kind="ExternalInput").ap()
    sn2k = nc.dram_tensor("sn2k", [128, NTKV], F32, kind="ExternalInput").ap()
    onesd = nc.dram_tensor("onesd", [128, 128], F32R, kind="ExternalInput").ap()
    out = nc.dram_tensor("out", [NTQ, D], F32, kind="ExternalOutput").ap()

    # blocked scratch
    qTsb = nc.dram_tensor("qTsb", [H, 2, 128, 512], F32R).ap()
    kTsb = nc.dram_tensor("kTsb", [H, 4, 128, 512], F32R).ap()
    vsb = nc.dram_tensor("vsb", [H, KC, 128, 128], F32R).ap()

    xT_r = xT.rearrange("(cc p) t -> cc p t", p=128)
    xTq_r = xTq.rearrange("(cc p) t -> cc p t", p=128)

    with tile.TileContext(nc) as tc, ExitStack() as octx:
        cs_pool = octx.enter_context(tc.tile_pool(name="cs", bufs=1))
        csq_sb = cs_pool.tile([128, NTQ], F32, tag="csq")
        snq_sb = cs_pool.tile([128, NTQ], F32, tag="snq")
        csk_sb = cs_pool.tile([128, NTKV], F32, tag="csk")
        snk_sb = cs_pool.tile([128, NTKV], F32, tag="snk")
        nc.sync.dma_start(csq_sb[:], cs2q[:])
        nc.sync.dma_start(snq_sb[:], sn2q[:])
        nc.sync.dma_start(csk_sb[:], cs2k[:])
        nc.sync.dma_start(snk_sb[:], sn2k[:])

        # ---------------- phase 1: QKV projections + RoPE ----------------
        with ExitStack() as p1:
            xt_pool = p1.enter_context(tc.tile_pool(name="xt", bufs=18))
            wqk_pool = p1.enter_context(tc.tile_pool(name="wqk", bufs=34))
            wv_pool = p1.enter_context(tc.tile_pool(name="wv", bufs=18))
            ev_pool = p1.enter_context(tc.tile_pool(name="ev", bufs=4))
            rp_pool = p1.enter_context(tc.tile_pool(name="rp", bufs=3))
            ps1 = p1.enter_context(tc.tile_pool(name="ps1", bufs=4, space="PSUM"))

            def rope_evict(ps, cs_sb, sn_sb, toff, dst_blk):
                # dst_blk: [128, 512] DRAM block (contiguous)
                sf = rp_pool.tile([128, 512], F32, tag="sf")
                nc.scalar.copy(sf[:], ps[:])
                sw = rp_pool.tile([128, 512], F32, tag="sw")
                nc.scalar.dma_start(sw[0:64, :], sf[64:128, :])
                nc.scalar.dma_start(sw[64:128, :], sf[0:64, :])
                ta = rp_pool.tile([128, 512], F32, tag="ta")
                nc.vector.tensor_mul(ta[:], sf[:], cs_sb[:, toff:toff + 512])
                tb = rp_pool.tile([128, 512], F32, tag="tb")
                nc.vector.tensor_mul(tb[:], sw[:], sn_sb[:, toff:toff + 512])
                o = ev_pool.tile([128, 512], F32R, tag="ev")
                nc.vector.tensor_add(o[:], ta[:], tb[:])
                nc.scalar.dma_start(dst_blk, o[:])

            # --- Q projection (local tokens) ---
            xq = []
            for cc in range(CC):
                t_ = xt_pool.tile([128, 1024], F32R, tag="xt")
                nc.sync.dma_start(t_[:], xTq_r[cc])
                xq.append(t_)
            for h in range(H):
                wsl = []
                for cc in range(CC):
                    w_ = wqk_pool.tile([128, 128], F32R, tag="wqk")
                    nc.sync.dma_start(w_[:], wqb[h, cc])
                    wsl.append(w_)
                for tt in range(NTQ // 512):
                    ps = ps1.tile([128, 512], F32, tag="ps1")
                    for cc in range(CC):
                        nc.tensor.matmul(ps[:], wsl[cc][:], xq[cc][:, ts(tt, 512)],
                                         start=(cc == 0), stop=(cc == CC - 1))
                    rope_evict(ps, csq_sb, snq_sb, tt * 512, qTsb[h, tt])

            # --- K^T and V per kv t-half ---
            for th in range(2):
                xk = []
                for cc in range(CC):
                    t_ = xt_pool.tile([128, 1024], F32R, tag="xt")
                    nc.sync.dma_start(t_[:], xT_r[cc, :, th * 1024:(th + 1) * 1024])
                    xk.append(t_)
                for h in range(H):
                    wsl = []
                    for cc in range(CC):
                        w_ = wqk_pool.tile([128, 128], F32R, tag="wqk")
                        nc.sync.dma_start(w_[:], wkb[h, cc])
                        wsl.append(w_)
                    for tt in range(2):
                        ps = ps1.tile([128, 512], F32, tag="ps1")
                        for cc in range(CC):
                            nc.tensor.matmul(ps[:], wsl[cc][:], xk[cc][:, ts(tt, 512)],
                                             start=(cc == 0), stop=(cc == CC - 1))
                        rope_evict(ps, csk_sb, snk_sb, th * 1024 + tt * 512,
                                   kTsb[h, th * 2 + tt])
                for ft in range(4):
                    wvl = []
                    for cc in range(CC):
                        w_ = wv_pool.tile([128, 512], F32R, tag="wv")
                        nc.sync.dma_start(w_[:], wvb[ft, cc])
                        wvl.append(w_)
                    for tch in range(8):
                        ps = ps1.tile([128, 512], F32, tag="ps1")
                        for cc in range(CC):
                            nc.tensor.matmul(ps[:], xk[cc][:, ts(tch, 128)], wvl[cc][:],
                                             start=(cc == 0), stop=(cc == CC - 1))
                        o = ev_pool.tile([128, 512], F32R, tag="ev")
                        nc.scalar.copy(o[:], ps[:])
                        for hh in range(4):
                            nc.scalar.dma_start(
                                vsb[ft * 4 + hh, th * 8 + tch],
                                o[:, hh * 128:(hh + 1) * 128])

        # ---------------- phases 2+3 ----------------
        with ExitStack() as p23:
            oT_pool = p23.enter_context(tc.tile_pool(name="oT", bufs=1))
            one_pool = p23.enter_context(tc.tile_pool(name="one", bufs=1))
            oT_all = oT_pool.tile([128, H, NTQ], F32R, tag="oT")
            ones128 = one_pool.tile([128, 1], F32R, tag="o128")
            nc.sync.dma_start(ones128[:], onesd[:, 0:1])
            ones1 = one_pool.tile([1, 128], F32R, tag="o1")
            nc.sync.dma_start(ones1[:], onesd[0:1, :])

            # ----- phase 2: SDPA per head -----
            with ExitStack() as p2:
                qh_pool = p2.enter_context(tc.tile_pool(name="qh", bufs=2))
                kh_pool = p2.enter_context(tc.tile_pool(name="kh", bufs=2))
                vh_pool = p2.enter_context(tc.tile_pool(name="vh", bufs=2))
                e_pool = p2.enter_context(tc.tile_pool(name="eT", bufs=4))
                es_pool = p2.enter_context(tc.tile_pool(name="es", bufs=3))
                rs_pool = p2.enter_context(tc.tile_pool(name="rs", bufs=3))
                s_ps_pool = p2.enter_context(
                    tc.tile_pool(name="sps", bufs=3, space="PSUM"))
                o_ps_pool = p2.enter_context(
                    tc.tile_pool(name="ops", bufs=2, space="PSUM"))
                m_ps_pool = p2.enter_context(
                    tc.tile_pool(name="mps", bufs=2, space="PSUM"))

                for h in range(H):
                    qh_sb = qh_pool.tile([128, NTQ], F32R, tag="qh")
                    for tt in range(2):
                        nc.sync.dma_start(qh_sb[:, ts(tt, 512)], qTsb[h, tt])
                    kh_sb = kh_pool.tile([128, NTKV], F32R, tag="kh")
                    for tt in range(4):
                        nc.sync.dma_start(kh_sb[:, ts(tt, 512)], kTsb[h, tt])
                    vh_sb = vh_pool.tile([128, KC, 128], F32R, tag="vh")
                    for kc in range(KC):
                        nc.sync.dma_start(vh_sb[:, kc, :], vsb[h, kc])
                    for qh in range(NTQ // 512):
                        qsl = qh_sb[:, ts(qh, 512)]
                        esum = es_pool.tile([128, 512], F32R, tag="es")
                        o_ps = o_ps_pool.tile([128, 512], F32, tag="ops")
                        for kc in range(KC):
                            s_ps = s_ps_pool.tile([128, 512], F32, tag="sps")
                            nc.tensor.matmul(s_ps[:], kh_sb[:, ts(kc, 128)], qsl,
                                             start=True, stop=True)
                            eT = e_pool.tile([128, 512], F32R, tag="eT")
                            nc.scalar.activation(eT[:], s_ps[:], EXP, scale=SCALE)
                            if kc == 0:
                                nc.vector.tensor_copy(esum[:], eT[:])
                            else:
                                nc.vector.tensor_add(esum[:], esum[:], eT[:])
                            nc.tensor.matmul(o_ps[:], vh_sb[:, kc, :], eT[:],
                                             start=(kc == 0), stop=(kc == KC - 1))
                        m_ps = m_ps_pool.tile([128, 512], F32, tag="mps")
                        nc.tensor.matmul(m_ps[0:1, :], ones128[:], esum[:],
                                         start=True, stop=True)
                        rs = rs_pool.tile([1, 512], F32R, tag="rs")
                        with nc.allow_low_precision(reason="f32r is 4-byte"):
                            nc.vector.reciprocal(rs[:], m_ps[0:1, :])
                        nc.tensor.matmul(m_ps[:, :], ones1[:], rs[:],
                                         start=True, stop=True)
                        bc_sb = rs_pool.tile([128, 512], F32, tag="bc")
                        nc.scalar.copy(bc_sb[:], m_ps[:])
                        nc.vector.tensor_mul(
                            oT_all[:, h, ts(qh, 512)], o_ps[:], bc_sb[:])

            # ----- phase 3: output projection -----
            with ExitStack() as p3:
                wp_pool = p3.enter_context(tc.tile_pool(name="wp", bufs=18))
                outev_pool = p3.enter_context(tc.tile_pool(name="outev", bufs=4))
                ps3 = p3.enter_context(tc.tile_pool(name="ps3", bufs=4, space="PSUM"))

                for ft in range(4):
                    wpl = []
                    for hc in range(CC):
                        w_ = wp_pool.tile([128, 512], F32R, tag="wp")
                        nc.sync.dma_start(w_[:], wpb[ft, hc])
                        wpl.append(w_)
                    for tch in range(NTQ // 128):
                        ps = ps3.tile([128, 512], F32, tag="ps3")
                        for hc in range(H):
                            nc.tensor.matmul(ps[:], oT_all[:, hc, ts(tch, 128)],
                                             wpl[hc][:],
                                             start=(hc == 0), stop=(hc == H - 1))
                        oev = outev_pool.tile([128, 512], F32, tag="outev")
                        nc.scalar.copy(oev[:], ps[:])
                        nc.scalar.dma_start(
                            out[ts(tch, 128), ft * 512:(ft + 1) * 512], oev[:])

    _split_multi_waits(nc)
    return nc


# ---------------------------------------------------------------------------
# host-side prep / assembly
# ---------------------------------------------------------------------------

_ONES = np.ones((128, 128), dtype=np.float32)


def prep_inputs(x, w_attn, w_proj):
    x = np.asarray(x, dtype=np.float32)
    w_attn = np.asarray(w_attn, dtype=np.float32)
    w_proj = np.asarray(w_proj, dtype=np.float32)

    perm = np.concatenate([np.arange(0, HD, 2), np.arange(1, HD, 2)])
    colperm = (np.arange(H)[:, None] * HD + perm[None, :]).ravel()

    wq, wk, wv = w_attn[0:D], w_attn[D:2 * D], w_attn[2 * D:3 * D]
    # blocked: [h, cc, 128, 128] from wT[c, f] (f head-permuted for q/k)
    wqb = np.ascontiguousarray(
        wq.T[:, colperm].reshape(CC, 128, H, 128).transpose(2, 0, 1, 3))
    wkb = np.ascontiguousarray(
        wk.T[:, colperm].reshape(CC, 128, H, 128).transpose(2, 0, 1, 3))
    wvb = np.ascontiguousarray(
        wv.T.reshape(CC, 128, 4, 512).transpose(2, 0, 1, 3))
    wpb = np.ascontiguousarray(
        w_proj.T.reshape(CC, 128, 4, 512).transpose(2, 0, 1, 3))

    inv = 1.0 / (10000.0 ** (np.arange(0, HD, 2, dtype=np.float64) / HD))
    fr = np.outer(np.arange(T, dtype=np.float64), inv)
    cos = np.cos(fr).T
    sin = np.sin(fr).T
    cs2 = np.concatenate([cos, cos], 0).astype(np.float32)
    sn2 = np.concatenate([-sin, sin], 0).astype(np.float32)

    in_maps = []
    for i in range(N_CORES):
        b, half = i // 2, i % 2
        q0 = half * NTQ
        xT_b = np.ascontiguousarray(x[b].T)
        in_maps.append({
            "xT": xT_b,
            "xTq": np.ascontiguousarray(xT_b[:, q0:q0 + NTQ]),
            "wqb": wqb, "wkb": wkb, "wvb": wvb, "wpb": wpb,
            "cs2q": np.ascontiguousarray(cs2[:, q0:q0 + NTQ]),
            "sn2q": np.ascontiguousarray(sn2[:, q0:q0 + NTQ]),
            "cs2k": cs2, "sn2k": sn2,
            "onesd": _ONES,
        })
    return in_maps


def assemble(results):
    out = np.empty((B, T, D), dtype=np.float32)
    for i in range(N_CORES):
        b, half = i // 2, i % 2
        out[b, half * NTQ:(half + 1) * NTQ, :] = results[i]["out"]
    return out


_nc_cache = None


def _get_nc():
    global _nc_cache
    if _nc_cache is None:
        _nc_cache = build_nc()
    return _nc_cache


def kernel(x, w_attn, w_proj):
    from concourse.bass_utils import run_bass_kernel_spmd
    nc = _get_nc()
    in_maps = prep_inputs(x, w_attn, w_proj)
    res = run_bass_kernel_spmd(nc, in_maps, list(range(N_CORES)))
    return assemble(res.results)


def run_profiled(x, w_attn, w_proj, trace_cores=None):
    """Like kernel() but with NTFF profiling; returns BassKernelResults."""
    from concourse.bass_utils import run_bass_kernel_spmd
    import sys as _sys, types as _types
    try:
        import antenv
        if "antenv.axon_hooks" not in _sys.modules:
            mod = _types.ModuleType("antenv.axon_hooks")
            _h = [None]
            mod.set_axon_ntff_profile_hook = lambda h: _h.__setitem__(0, h)
            mod.get_axon_ntff_profile_hook = lambda: _h[0]
            _sys.modules["antenv.axon_hooks"] = mod
            antenv.axon_hooks = mod
            from trn_agent_boot.trn_boot import _ntff_profile_via_ctypes
            mod.set_axon_ntff_profile_hook(
                _ntff_profile_via_ctypes('/opt/axon/libaxon_pjrt.so'))
    except Exception as e:  # profiling is best-effort
        print("profile hook setup failed:", e)
    nc = _get_nc()
    in_maps = prep_inputs(x, w_attn, w_proj)
    return run_bass_kernel_spmd(
        nc, in_maps, list(range(N_CORES)), trace=True,
        trace_cores=trace_cores if trace_cores is not None else [0])


# revision 6
# speedup vs baseline: 1.2896x; 1.1046x over previous
"""8-core Trainium2 Bass kernel for nn_Attention_89489938579587.

reference: qkv = x @ w_attn.T; split q,k,v per 16 heads (HD=128); RoPE
(interleaved pairs); non-causal SDPA; y @ w_proj.T.  B=4, T=2048, D=2048.

Sharding: core i -> batch b=i//2, query-token half i%2 (1024 tokens).
Each core computes QKV for its batch (K,V over all 2048 kv tokens, Q over
its 1024), RoPE, SDPA for all 16 heads over its queries, and the output
projection rows for its tokens.  Host concatenates 8 x [1024, 2048].
No collectives.  Matmuls run in float32r (full PE rate, ~1e-4 rel err).

kv tokens are processed LOCAL-half-first (softmax is key-order invariant;
the RoPE tables are host-reordered to match), so the local x tiles are
shared between the Q pass and the first KV pass.  Weights are supplied in
partition-major slab layouts so each head/f-tile slab is one large-segment
DMA.  Attention outputs stay resident in SBUF.

Self-contained: builds the Bass program on first call, runs via
run_bass_kernel_spmd on cores 0-7.
"""

import numpy as np
from contextlib import ExitStack

import concourse.bass as bass
import concourse.tile as tile
from concourse import mybir
from concourse.bass import ts

# ---------------------------------------------------------------------------
# Workarounds for this toolchain:
# 1) walrus here rejects any instruction with >1 semaphore wait ("Too many
#    sync wait commands").  After Tile lowering, split extra waits onto
#    same-engine InstNoOp instructions inserted right before the offender.
# 2) the Tile tail drain carries many waits; patch _drain_and_barrier to put
#    them on SP nops (one each) before a waitless drain.
# ---------------------------------------------------------------------------
import bass_rust


def _split_multi_waits(nc, max_waits=1):
    n = 0
    for fn in nc.m.functions:
        for blk in fn.blocks:
            insts = blk.instructions
            i = 0
            while i < len(insts):
                inst = insts[i]
                si = inst.sync_info
                waits = list(si.on_wait) if (si is not None and si.on_wait) else []
                if len(waits) > max_waits:
                    si.on_wait = waits[:max_waits]
                    extra = waits[max_waits:]
                    for j in range(0, len(extra), max_waits):
                        nop = mybir.InstNoOp(
                            name=nc.get_next_instruction_name(), ins=[], outs=[])
                        nop.engine = inst.engine
                        nop.sync_info = bass_rust.SyncInfo(
                            on_wait=extra[j:j + max_waits], on_update=[])
                        nc.register_instruction(nop, overwrite=True)
                        insts.insert(i, nop)
                        i += 1
                        n += 1
                i += 1
    return n


def _patched_drain_and_barrier(self, tick_clock, wait_clock):
    from concourse.vector_clock import ScopedClock
    nc = self.nc
    probe = nc.sync.nop()
    wait_clock.add_sem_waits(probe.ins, ScopedClock({None: tick_clock.global_clock}))
    si = probe.ins.sync_info
    waits = list(si.on_wait or []) if si is not None else []
    if len(waits) > 1:
        si.on_wait = [waits[0]]
        for w in waits[1:]:
            nop = nc.sync.nop()
            nsi = nop.ins.sync_info
            if nsi is None:
                nop.ins.sync_info = bass_rust.SyncInfo(on_wait=[w], on_update=[])
            else:
                nsi.on_wait = [w]
    nc.sync.drain()
    nc.all_engine_barrier()
    assert self.sems is not None
    popped = nc._tile_sem_poison_stack.pop()
    assert popped is self._sem_poison
    nc.clear_and_free_semaphores(list(self.sems.allocated().values()))
    nc.all_engine_barrier()


_patched = False


def _apply_patches():
    global _patched
    if not _patched:
        tile.TileContext._drain_and_barrier = _patched_drain_and_barrier
        _patched = True


# ---------------------------------------------------------------------------
# Problem constants (hardcoded per spec)
# ---------------------------------------------------------------------------
F32R = mybir.dt.float32r
F32 = mybir.dt.float32
EXP = mybir.ActivationFunctionType.Exp

B, T, D, H, HD = 4, 2048, 2048, 16, 128
CC = D // 128           # contraction chunks
NTQ = 1024              # query tokens per core
NTKV = T                # kv tokens per core
KC = NTKV // 128        # kv chunks
SCALE = 1.0 / float(np.sqrt(HD))
N_CORES = 8


def build_nc(n_cores=N_CORES):
    _apply_patches()
    nc = bass.Bass("TRN2", target_bir_lowering=False, debug=False,
                   num_devices=n_cores)
    # x columns: local query half / remote half (kv order = local;remote)
    xTq = nc.dram_tensor("xTq", [D, NTQ], F32R, kind="ExternalInput").ap()
    xTr = nc.dram_tensor("xTr", [D, NTQ], F32R, kind="ExternalInput").ap()
    # partition-major weight slabs
    wqs = nc.dram_tensor("wqs", [H, 128, CC * 128], F32R, kind="ExternalInput").ap()
    wks = nc.dram_tensor("wks", [H, 128, CC * 128], F32R, kind="ExternalInput").ap()
    wvs = nc.dram_tensor("wvs", [4, 2, 128, 8 * 512], F32R, kind="ExternalInput").ap()
    wps = nc.dram_tensor("wps", [4, 2, 128, 8 * 512], F32R, kind="ExternalInput").ap()
    cs2q = nc.dram_tensor("cs2q", [128, NTQ], F32, kind="ExternalInput").ap()
    sn2q = nc.dram_tensor("sn2q", [128, NTQ], F32, kind="ExternalInput").ap()
    cs2k = nc.dram_tensor("cs2k", [128, NTKV], F32, kind="ExternalInput").ap()
    sn2k = nc.dram_tensor("sn2k", [128, NTKV], F32, kind="ExternalInput").ap()
    onesd = nc.dram_tensor("onesd", [128, 128], F32R, kind="ExternalInput").ap()
    out = nc.dram_tensor("out", [NTQ, D], F32, kind="ExternalOutput").ap()

    # scratch: row-contiguous q/k; partition-major v
    qTs = nc.dram_tensor("qTs", [D, NTQ], F32R).ap()
    kTs = nc.dram_tensor("kTs", [D, NTKV], F32R).ap()
    vsc = nc.dram_tensor("vsc", [H, 128, KC * 128], F32R).ap()

    xTq_r = xTq.rearrange("(cc p) t -> cc p t", p=128)
    xTr_r = xTr.rearrange("(cc p) t -> cc p t", p=128)

    with tile.TileContext(nc) as tc, ExitStack() as octx:
        cs_pool = octx.enter_context(tc.tile_pool(name="cs", bufs=1))
        csq_sb = cs_pool.tile([128, NTQ], F32, tag="csq")
        snq_sb = cs_pool.tile([128, NTQ], F32, tag="snq")
        csk_sb = cs_pool.tile([128, NTKV], F32, tag="csk")
        snk_sb = cs_pool.tile([128, NTKV], F32, tag="snk")
        nc.sync.dma_start(csq_sb[:], cs2q[:])
        nc.sync.dma_start(snq_sb[:], sn2q[:])
        nc.sync.dma_start(csk_sb[:], cs2k[:])
        nc.sync.dma_start(snk_sb[:], sn2k[:])

        # ---------------- phase 1: QKV projections + RoPE ----------------
        with ExitStack() as p1:
            xt_pool = p1.enter_context(tc.tile_pool(name="xt", bufs=20))
            wqk_pool = p1.enter_context(tc.tile_pool(name="wqk", bufs=3))
            wv_pool = p1.enter_context(tc.tile_pool(name="wv", bufs=2))
            ev_pool = p1.enter_context(tc.tile_pool(name="ev", bufs=4))
            rp_pool = p1.enter_context(tc.tile_pool(name="rp", bufs=3))
            ps1 = p1.enter_context(tc.tile_pool(name="ps1", bufs=4, space="PSUM"))

            def rope_evict(ps, cs_sb, sn_sb, toff, dst):
                sf = rp_pool.tile([128, 512], F32, tag="sf")
                nc.scalar.copy(sf[:], ps[:])
                sw = rp_pool.tile([128, 512], F32, tag="sw")
                nc.scalar.dma_start(sw[0:64, :], sf[64:128, :])
                nc.scalar.dma_start(sw[64:128, :], sf[0:64, :])
                ta = rp_pool.tile([128, 512], F32, tag="ta")
                nc.vector.tensor_mul(ta[:], sf[:], cs_sb[:, toff:toff + 512])
                tb = rp_pool.tile([128, 512], F32, tag="tb")
                nc.vector.tensor_mul(tb[:], sw[:], sn_sb[:, toff:toff + 512])
                o = ev_pool.tile([128, 512], F32R, tag="ev")
                nc.vector.tensor_add(o[:], ta[:], tb[:])
                nc.scalar.dma_start(dst, o[:])

            # local x tiles (used by Q pass AND kv pass 0)
            xq = []
            for cc in range(CC):
                t_ = xt_pool.tile([128, 1024], F32R, tag="xt")
                nc.sync.dma_start(t_[:], xTq_r[cc])
                xq.append(t_)

            # --- Q projection ---
            for h in range(H):
                wsl = wqk_pool.tile([128, CC, 128], F32R, tag="wqk")
                nc.sync.dma_start(wsl[:], wqs[h])
                for tt in range(NTQ // 512):
                    ps = ps1.tile([128, 512], F32, tag="ps1")
                    for cc in range(CC):
                        nc.tensor.matmul(ps[:], wsl[:, cc, :], xq[cc][:, ts(tt, 512)],
                                         start=(cc == 0), stop=(cc == CC - 1))
                    rope_evict(ps, csq_sb, snq_sb, tt * 512,
                               qTs[h * 128:(h + 1) * 128, tt * 512:(tt + 1) * 512])

            # --- K^T and V per kv t-half (0 = local tokens, 1 = remote) ---
            for th in range(2):
                if th == 0:
                    xk = xq
                else:
                    xk = []
                    for cc in range(CC):
                        t_ = xt_pool.tile([128, 1024], F32R, tag="xt")
                        nc.sync.dma_start(t_[:], xTr_r[cc])
                        xk.append(t_)
                for h in range(H):
                    wsl = wqk_pool.tile([128, CC, 128], F32R, tag="wqk")
                    nc.sync.dma_start(wsl[:], wks[h])
                    for tt in range(2):
                        ps = ps1.tile([128, 512], F32, tag="ps1")
                        for cc in range(CC):
                            nc.tensor.matmul(ps[:], wsl[:, cc, :], xk[cc][:, ts(tt, 512)],
                                             start=(cc == 0), stop=(cc == CC - 1))
                        toff = th * 1024 + tt * 512
                        rope_evict(ps, csk_sb, snk_sb, toff,
                                   kTs[h * 128:(h + 1) * 128, toff:toff + 512])
                for ft in range(4):
                    wvl0 = wv_pool.tile([128, 8, 512], F32R, tag="wv")
                    nc.sync.dma_start(wvl0[:], wvs[ft, 0])
                    wvl1 = wv_pool.tile([128, 8, 512], F32R, tag="wv")
                    nc.sync.dma_start(wvl1[:], wvs[ft, 1])
                    for tch in range(8):
                        ps = ps1.tile([128, 512], F32, tag="ps1")
                        for cc in range(CC):
                            wv_ap = (wvl0 if cc < 8 else wvl1)[:, cc % 8, :]
                            nc.tensor.matmul(ps[:], xk[cc][:, ts(tch, 128)], wv_ap,
                                             start=(cc == 0), stop=(cc == CC - 1))
                        o = ev_pool.tile([128, 512], F32R, tag="ev")
                        nc.scalar.copy(o[:], ps[:])
                        kc = th * 8 + tch
                        for hh in range(4):
                            nc.scalar.dma_start(
                                vsc[ft * 4 + hh, :, kc * 128:(kc + 1) * 128],
                                o[:, hh * 128:(hh + 1) * 128])

        # ---------------- phases 2+3 ----------------
        with ExitStack() as p23:
            oT_pool = p23.enter_context(tc.tile_pool(name="oT", bufs=1))
            one_pool = p23.enter_context(tc.tile_pool(name="one", bufs=1))
            oT_all = oT_pool.tile([128, H, NTQ], F32R, tag="oT")
            ones128 = one_pool.tile([128, 1], F32R, tag="o128")
            nc.sync.dma_start(ones128[:], onesd[:, 0:1])
            ones1 = one_pool.tile([1, 128], F32R, tag="o1")
            nc.sync.dma_start(ones1[:], onesd[0:1, :])

            # ----- phase 2: SDPA per head, epilogue software-pipelined -----
            with ExitStack() as p2:
                qh_pool = p2.enter_context(tc.tile_pool(name="qh", bufs=2))
                kh_pool = p2.enter_context(tc.tile_pool(name="kh", bufs=2))
                vh_pool = p2.enter_context(tc.tile_pool(name="vh", bufs=2))
                e_pool = p2.enter_context(tc.tile_pool(name="eT", bufs=4))
                es_pool = p2.enter_context(tc.tile_pool(name="es", bufs=3))
                rs_pool = p2.enter_context(tc.tile_pool(name="rs", bufs=3))
                s_ps_pool = p2.enter_context(
                    tc.tile_pool(name="sps", bufs=3, space="PSUM"))
                o_ps_pool = p2.enter_context(
                    tc.tile_pool(name="ops", bufs=3, space="PSUM"))
                m_ps_pool = p2.enter_context(
                    tc.tile_pool(name="mps", bufs=2, space="PSUM"))

                pending = []

                def emit_epilogue():
                    if not pending:
                        return
                    h, qh, esum, o_ps = pending.pop(0)
                    m_ps = m_ps_pool.tile([128, 512], F32, tag="mps")
                    nc.tensor.matmul(m_ps[0:1, :], ones128[:], esum[:],
                                     start=True, stop=True)
                    rs = rs_pool.tile([1, 512], F32R, tag="rs")
                    with nc.allow_low_precision(reason="f32r is 4-byte"):
                        nc.vector.reciprocal(rs[:], m_ps[0:1, :])
                    nc.tensor.matmul(m_ps[:, :], ones1[:], rs[:],
                                     start=True, stop=True)
                    bc_sb = rs_pool.tile([128, 512], F32, tag="bc")
                    nc.scalar.copy(bc_sb[:], m_ps[:])
                    nc.vector.tensor_mul(
                        oT_all[:, h, ts(qh, 512)], o_ps[:], bc_sb[:])

                for h in range(H):
                    qh_sb = qh_pool.tile([128, NTQ], F32R, tag="qh")
                    nc.sync.dma_start(qh_sb[:], qTs[h * 128:(h + 1) * 128, :])
                    kh_sb = kh_pool.tile([128, NTKV], F32R, tag="kh")
                    nc.sync.dma_start(kh_sb[:], kTs[h * 128:(h + 1) * 128, :])
                    vh_sb = vh_pool.tile([128, KC * 128], F32R, tag="vh")
                    nc.sync.dma_start(vh_sb[:], vsc[h])
                    for qh in range(NTQ // 512):
                        qsl = qh_sb[:, ts(qh, 512)]
                        esum = es_pool.tile([128, 512], F32R, tag="es")
                        o_ps = o_ps_pool.tile([128, 512], F32, tag="ops")
                        for kc in range(KC):
                            s_ps = s_ps_pool.tile([128, 512], F32, tag="sps")
                            nc.tensor.matmul(s_ps[:], kh_sb[:, ts(kc, 128)], qsl,
                                             start=True, stop=True)
                            eT = e_pool.tile([128, 512], F32R, tag="eT")
                            nc.scalar.activation(eT[:], s_ps[:], EXP, scale=SCALE)
                            if kc == 0:
                                nc.vector.tensor_copy(esum[:], eT[:])
                            else:
                                nc.vector.tensor_add(esum[:], esum[:], eT[:])
                            nc.tensor.matmul(o_ps[:], vh_sb[:, ts(kc, 128)], eT[:],
                                             start=(kc == 0), stop=(kc == KC - 1))
                        pending.append((h, qh, esum, o_ps))
                        if len(pending) > 1:
                            emit_epilogue()
                while pending:
                    emit_epilogue()

            # ----- phase 3: output projection -----
            with ExitStack() as p3:
                wp_pool = p3.enter_context(tc.tile_pool(name="wp", bufs=2))
                outev_pool = p3.enter_context(tc.tile_pool(name="outev", bufs=4))
                ps3 = p3.enter_context(tc.tile_pool(name="ps3", bufs=4, space="PSUM"))

                for ft in range(4):
                    wpl0 = wp_pool.tile([128, 8, 512], F32R, tag="wp")
                    nc.sync.dma_start(wpl0[:], wps[ft, 0])
                    wpl1 = wp_pool.tile([128, 8, 512], F32R, tag="wp")
                    nc.sync.dma_start(wpl1[:], wps[ft, 1])
                    for tch in range(NTQ // 128):
                        ps = ps3.tile([128, 512], F32, tag="ps3")
                        for hc in range(H):
                            wp_ap = (wpl0 if hc < 8 else wpl1)[:, hc % 8, :]
                            nc.tensor.matmul(ps[:], oT_all[:, hc, ts(tch, 128)],
                                             wp_ap,
                                             start=(hc == 0), stop=(hc == H - 1))
                        oev = outev_pool.tile([128, 512], F32, tag="outev")
                        nc.scalar.copy(oev[:], ps[:])
                        nc.scalar.dma_start(
                            out[ts(tch, 128), ft * 512:(ft + 1) * 512], oev[:])

    _split_multi_waits(nc)
    return nc


# ---------------------------------------------------------------------------
# host-side prep / assembly
# ---------------------------------------------------------------------------

_ONES = np.ones((128, 128), dtype=np.float32)


def prep_inputs(x, w_attn, w_proj):
    x = np.asarray(x, dtype=np.float32)
    w_attn = np.asarray(w_attn, dtype=np.float32)
    w_proj = np.asarray(w_proj, dtype=np.float32)

    perm = np.concatenate([np.arange(0, HD, 2), np.arange(1, HD, 2)])
    colperm = (np.arange(H)[:, None] * HD + perm[None, :]).ravel()

    wq, wk, wv = w_attn[0:D], w_attn[D:2 * D], w_attn[2 * D:3 * D]
    # partition-major slabs: [h, p, cc*128] with wT[c, f] = w.T
    wqs = np.ascontiguousarray(
        wq.T[:, colperm].reshape(CC, 128, H, 128)
        .transpose(2, 1, 0, 3).reshape(H, 128, CC * 128))
    wks = np.ascontiguousarray(
        wk.T[:, colperm].reshape(CC, 128, H, 128)
        .transpose(2, 1, 0, 3).reshape(H, 128, CC * 128))
    # [ft, half, p, 8*512]
    wvs = np.ascontiguousarray(
        wv.T.reshape(2, 8, 128, 4, 512)
        .transpose(3, 0, 2, 1, 4).reshape(4, 2, 128, 8 * 512))
    wps = np.ascontiguousarray(
        w_proj.T.reshape(2, 8, 128, 4, 512)
        .transpose(3, 0, 2, 1, 4).reshape(4, 2, 128, 8 * 512))

    inv = 1.0 / (10000.0 ** (np.arange(0, HD, 2, dtype=np.float64) / HD))
    fr = np.outer(np.arange(T, dtype=np.float64), inv)
    cos = np.cos(fr).T
    sin = np.sin(fr).T
    cs2 = np.concatenate([cos, cos], 0).astype(np.float32)
    sn2 = np.concatenate([-sin, sin], 0).astype(np.float32)

    in_maps = []
    for i in range(N_CORES):
        b, half = i // 2, i % 2
        q0 = half * NTQ
        r0 = (1 - half) * NTQ
        xT_b = np.ascontiguousarray(x[b].T)
        # kv order: local half first, then remote half
        cs2k = np.concatenate([cs2[:, q0:q0 + NTQ], cs2[:, r0:r0 + NTQ]], axis=1)
        sn2k = np.concatenate([sn2[:, q0:q0 + NTQ], sn2[:, r0:r0 + NTQ]], axis=1)
        in_maps.append({
            "xTq": np.ascontiguousarray(xT_b[:, q0:q0 + NTQ]),
            "xTr": np.ascontiguousarray(xT_b[:, r0:r0 + NTQ]),
            "wqs": wqs, "wks": wks, "wvs": wvs, "wps": wps,
            "cs2q": np.ascontiguousarray(cs2[:, q0:q0 + NTQ]),
            "sn2q": np.ascontiguousarray(sn2[:, q0:q0 + NTQ]),
            "cs2k": np.ascontiguousarray(cs2k),
            "sn2k": np.ascontiguousarray(sn2k),
            "onesd": _ONES,
        })
    return in_maps


def assemble(results):
    out = np.empty((B, T, D), dtype=np.float32)
    for i in range(N_CORES):
        b, half = i // 2, i % 2
        out[b, half * NTQ:(half + 1) * NTQ, :] = results[i]["out"]
    return out


_nc_cache = None


def _get_nc():
    global _nc_cache
    if _nc_cache is None:
        _nc_cache = build_nc()
    return _nc_cache


def kernel(x, w_attn, w_proj):
    from concourse.bass_utils import run_bass_kernel_spmd
    nc = _get_nc()
    in_maps = prep_inputs(x, w_attn, w_proj)
    res = run_bass_kernel_spmd(nc, in_maps, list(range(N_CORES)))
    return assemble(res.results)


def run_profiled(x, w_attn, w_proj, trace_cores=None):
    """Like kernel() but with NTFF profiling; returns BassKernelResults."""
    from concourse.bass_utils import run_bass_kernel_spmd
    import sys as _sys, types as _types
    try:
        import antenv
        if "antenv.axon_hooks" not in _sys.modules:
            mod = _types.ModuleType("antenv.axon_hooks")
            _h = [None]
            mod.set_axon_ntff_profile_hook = lambda h: _h.__setitem__(0, h)
            mod.get_axon_ntff_profile_hook = lambda: _h[0]
            _sys.modules["antenv.axon_hooks"] = mod
            antenv.axon_hooks = mod
            from trn_agent_boot.trn_boot import _ntff_profile_via_ctypes
            mod.set_axon_ntff_profile_hook(
                _ntff_profile_via_ctypes('/opt/axon/libaxon_pjrt.so'))
    except Exception as e:  # profiling is best-effort
        print("profile hook setup failed:", e)
    nc = _get_nc()
    in_maps = prep_inputs(x, w_attn, w_proj)
    return run_bass_kernel_spmd(
        nc, in_maps, list(range(N_CORES)), trace=True,
        trace_cores=trace_cores if trace_cores is not None else [0])


# revision 7
# speedup vs baseline: 1.3728x; 1.0645x over previous
"""8-core Trainium2 Bass kernel for nn_Attention_89489938579587.

reference: qkv = x @ w_attn.T; split q,k,v per 16 heads (HD=128); RoPE
(interleaved pairs); non-causal SDPA; y @ w_proj.T.  B=4, T=2048, D=2048.

Sharding: core i -> batch b=i//2, query-token half i%2 (1024 tokens).
Each core computes QKV for its batch (K,V over all 2048 kv tokens, Q over
its 1024), RoPE, SDPA for all 16 heads over its queries, and the output
projection rows for its tokens.  Host concatenates 8 x [1024, 2048].
No collectives.  Matmuls run in float32r (full PE rate, ~1e-4 rel err).

kv tokens are processed LOCAL-half-first (softmax is key-order invariant;
the RoPE tables are host-reordered to match), so the local x tiles are
shared between the Q pass and the first KV pass.  Weights are supplied in
partition-major slab layouts so each head/f-tile slab is one large-segment
DMA.  Attention outputs stay resident in SBUF.

Self-contained: builds the Bass program on first call, runs via
run_bass_kernel_spmd on cores 0-7.
"""

import numpy as np
from contextlib import ExitStack

import concourse.bass as bass
import concourse.tile as tile
from concourse import mybir
from concourse.bass import ts

# ---------------------------------------------------------------------------
# Workarounds for this toolchain:
# 1) walrus here rejects any instruction with >1 semaphore wait ("Too many
#    sync wait commands").  After Tile lowering, split extra waits onto
#    same-engine InstNoOp instructions inserted right before the offender.
# 2) the Tile tail drain carries many waits; patch _drain_and_barrier to put
#    them on SP nops (one each) before a waitless drain.
# ---------------------------------------------------------------------------
import bass_rust


def _split_multi_waits(nc, max_waits=1):
    n = 0
    for fn in nc.m.functions:
        for blk in fn.blocks:
            insts = blk.instructions
            i = 0
            while i < len(insts):
                inst = insts[i]
                si = inst.sync_info
                waits = list(si.on_wait) if (si is not None and si.on_wait) else []
                if len(waits) > max_waits:
                    si.on_wait = waits[:max_waits]
                    extra = waits[max_waits:]
                    for j in range(0, len(extra), max_waits):
                        nop = mybir.InstNoOp(
                            name=nc.get_next_instruction_name(), ins=[], outs=[])
                        nop.engine = inst.engine
                        nop.sync_info = bass_rust.SyncInfo(
                            on_wait=extra[j:j + max_waits], on_update=[])
                        nc.register_instruction(nop, overwrite=True)
                        insts.insert(i, nop)
                        i += 1
                        n += 1
                i += 1
    return n


def _patched_drain_and_barrier(self, tick_clock, wait_clock):
    from concourse.vector_clock import ScopedClock
    nc = self.nc
    probe = nc.sync.nop()
    wait_clock.add_sem_waits(probe.ins, ScopedClock({None: tick_clock.global_clock}))
    si = probe.ins.sync_info
    waits = list(si.on_wait or []) if si is not None else []
    if len(waits) > 1:
        si.on_wait = [waits[0]]
        for w in waits[1:]:
            nop = nc.sync.nop()
            nsi = nop.ins.sync_info
            if nsi is None:
                nop.ins.sync_info = bass_rust.SyncInfo(on_wait=[w], on_update=[])
            else:
                nsi.on_wait = [w]
    nc.sync.drain()
    nc.all_engine_barrier()
    assert self.sems is not None
    popped = nc._tile_sem_poison_stack.pop()
    assert popped is self._sem_poison
    nc.clear_and_free_semaphores(list(self.sems.allocated().values()))
    nc.all_engine_barrier()


_patched = False


def _apply_patches():
    global _patched
    if not _patched:
        tile.TileContext._drain_and_barrier = _patched_drain_and_barrier
        _patched = True


# ---------------------------------------------------------------------------
# Problem constants (hardcoded per spec)
# ---------------------------------------------------------------------------
F32R = mybir.dt.float32r
F32 = mybir.dt.float32
EXP = mybir.ActivationFunctionType.Exp

B, T, D, H, HD = 4, 2048, 2048, 16, 128
CC = D // 128           # contraction chunks
NTQ = 1024              # query tokens per core
NTKV = T                # kv tokens per core
KC = NTKV // 128        # kv chunks
SCALE = 1.0 / float(np.sqrt(HD))
N_CORES = 8


def build_nc(n_cores=N_CORES):
    _apply_patches()
    nc = bass.Bass("TRN2", target_bir_lowering=False, debug=False,
                   num_devices=n_cores)
    # x columns: local query half / remote half (kv order = local;remote)
    xTq = nc.dram_tensor("xTq", [D, NTQ], F32R, kind="ExternalInput").ap()
    xTr = nc.dram_tensor("xTr", [D, NTQ], F32R, kind="ExternalInput").ap()
    # partition-major weight slabs
    wqs = nc.dram_tensor("wqs", [H, 128, CC * 128], F32R, kind="ExternalInput").ap()
    wks = nc.dram_tensor("wks", [H, 128, CC * 128], F32R, kind="ExternalInput").ap()
    wvs = nc.dram_tensor("wvs", [4, 4, 128, 4 * 512], F32R, kind="ExternalInput").ap()
    wps = nc.dram_tensor("wps", [4, 4, 128, 4 * 512], F32R, kind="ExternalInput").ap()
    cs2q = nc.dram_tensor("cs2q", [128, NTQ], F32, kind="ExternalInput").ap()
    sn2q = nc.dram_tensor("sn2q", [128, NTQ], F32, kind="ExternalInput").ap()
    cs2k = nc.dram_tensor("cs2k", [128, NTKV], F32, kind="ExternalInput").ap()
    sn2k = nc.dram_tensor("sn2k", [128, NTKV], F32, kind="ExternalInput").ap()
    onesd = nc.dram_tensor("onesd", [128, 128], F32R, kind="ExternalInput").ap()
    out = nc.dram_tensor("out", [NTQ, D], F32, kind="ExternalOutput").ap()

    # scratch: row-contiguous q/k; partition-major v
    qTs = nc.dram_tensor("qTs", [D, NTQ], F32R).ap()
    kTs = nc.dram_tensor("kTs", [D, NTKV], F32R).ap()
    vsc = nc.dram_tensor("vsc", [H, 128, KC * 128], F32R).ap()

    xTq_r = xTq.rearrange("(cc p) t -> cc p t", p=128)
    xTr_r = xTr.rearrange("(cc p) t -> cc p t", p=128)

    with tile.TileContext(nc) as tc, ExitStack() as octx:
        cs_pool = octx.enter_context(tc.tile_pool(name="cs", bufs=1))
        csq_sb = cs_pool.tile([128, NTQ], F32, tag="csq")
        snq_sb = cs_pool.tile([128, NTQ], F32, tag="snq")
        csk_sb = cs_pool.tile([128, NTKV], F32, tag="csk")
        snk_sb = cs_pool.tile([128, NTKV], F32, tag="snk")
        nc.sync.dma_start(csq_sb[:], cs2q[:])
        nc.sync.dma_start(snq_sb[:], sn2q[:])
        nc.sync.dma_start(csk_sb[:], cs2k[:])
        nc.sync.dma_start(snk_sb[:], sn2k[:])

        # ---------------- phase 1: QKV projections + RoPE ----------------
        with ExitStack() as p1:
            xt_pool = p1.enter_context(tc.tile_pool(name="xt", bufs=20))
            wqk_pool = p1.enter_context(tc.tile_pool(name="wqk", bufs=3))
            wv_pool = p1.enter_context(tc.tile_pool(name="wv", bufs=6))
            ev_pool = p1.enter_context(tc.tile_pool(name="ev", bufs=4))
            rp_pool = p1.enter_context(tc.tile_pool(name="rp", bufs=2))
            ps1 = p1.enter_context(tc.tile_pool(name="ps1", bufs=4, space="PSUM"))

            def rope_evict(ps, cs_sb, sn_sb, toff, dst):
                sf = rp_pool.tile([128, 512], F32, tag="sf")
                nc.scalar.copy(sf[:], ps[:])
                sw = rp_pool.tile([128, 512], F32, tag="sw")
                nc.gpsimd.dma_start(sw[0:64, :], sf[64:128, :])
                nc.gpsimd.dma_start(sw[64:128, :], sf[0:64, :])
                ta = rp_pool.tile([128, 512], F32, tag="ta")
                nc.vector.tensor_mul(ta[:], sf[:], cs_sb[:, toff:toff + 512])
                tb = rp_pool.tile([128, 512], F32, tag="tb")
                nc.vector.tensor_mul(tb[:], sw[:], sn_sb[:, toff:toff + 512])
                o = ev_pool.tile([128, 512], F32R, tag="ev")
                nc.vector.tensor_add(o[:], ta[:], tb[:])
                nc.gpsimd.dma_start(dst, o[:])

            # local x tiles (used by Q pass AND kv pass 0)
            xq = []
            for cc in range(CC):
                t_ = xt_pool.tile([128, 1024], F32R, tag="xt")
                nc.sync.dma_start(t_[:], xTq_r[cc])
                xq.append(t_)

            # --- Q projection ---
            for h in range(H):
                wsl = wqk_pool.tile([128, CC, 128], F32R, tag="wqk")
                nc.sync.dma_start(wsl[:], wqs[h])
                for tt in range(NTQ // 512):
                    ps = ps1.tile([128, 512], F32, tag="ps1")
                    for cc in range(CC):
                        nc.tensor.matmul(ps[:], wsl[:, cc, :], xq[cc][:, ts(tt, 512)],
                                         start=(cc == 0), stop=(cc == CC - 1))
                    rope_evict(ps, csq_sb, snq_sb, tt * 512,
                               qTs[h * 128:(h + 1) * 128, tt * 512:(tt + 1) * 512])

            # --- K^T and V per kv t-half (0 = local tokens, 1 = remote) ---
            for th in range(2):
                if th == 0:
                    xk = xq
                else:
                    xk = []
                    for cc in range(CC):
                        t_ = xt_pool.tile([128, 1024], F32R, tag="xt")
                        nc.sync.dma_start(t_[:], xTr_r[cc])
                        xk.append(t_)
                for h in range(H):
                    wsl = wqk_pool.tile([128, CC, 128], F32R, tag="wqk")
                    nc.sync.dma_start(wsl[:], wks[h])
                    for tt in range(2):
                        ps = ps1.tile([128, 512], F32, tag="ps1")
                        for cc in range(CC):
                            nc.tensor.matmul(ps[:], wsl[:, cc, :], xk[cc][:, ts(tt, 512)],
                                             start=(cc == 0), stop=(cc == CC - 1))
                        toff = th * 1024 + tt * 512
                        rope_evict(ps, csk_sb, snk_sb, toff,
                                   kTs[h * 128:(h + 1) * 128, toff:toff + 512])
                for ft in range(4):
                    wvl = []
                    for qt in range(4):
                        w_ = wv_pool.tile([128, 4, 512], F32R, tag="wv")
                        nc.sync.dma_start(w_[:], wvs[ft, qt])
                        wvl.append(w_)
                    for tch in range(8):
                        ps = ps1.tile([128, 512], F32, tag="ps1")
                        for cc in range(CC):
                            wv_ap = wvl[cc // 4][:, cc % 4, :]
                            nc.tensor.matmul(ps[:], xk[cc][:, ts(tch, 128)], wv_ap,
                                             start=(cc == 0), stop=(cc == CC - 1))
                        o = ev_pool.tile([128, 512], F32R, tag="ev")
                        nc.scalar.copy(o[:], ps[:])
                        kc = th * 8 + tch
                        for hh in range(4):
                            nc.gpsimd.dma_start(
                                vsc[ft * 4 + hh, :, kc * 128:(kc + 1) * 128],
                                o[:, hh * 128:(hh + 1) * 128])

        # ---------------- phases 2+3 ----------------
        with ExitStack() as p23:
            oT_pool = p23.enter_context(tc.tile_pool(name="oT", bufs=1))
            one_pool = p23.enter_context(tc.tile_pool(name="one", bufs=1))
            oT_all = oT_pool.tile([128, H, NTQ], F32R, tag="oT")
            ones128 = one_pool.tile([128, 1], F32R, tag="o128")
            nc.sync.dma_start(ones128[:], onesd[:, 0:1])
            ones1 = one_pool.tile([1, 128], F32R, tag="o1")
            nc.sync.dma_start(ones1[:], onesd[0:1, :])

            # ----- phase 2: SDPA per head, epilogue software-pipelined -----
            with ExitStack() as p2:
                qh_pool = p2.enter_context(tc.tile_pool(name="qh", bufs=2))
                kh_pool = p2.enter_context(tc.tile_pool(name="kh", bufs=2))
                vh_pool = p2.enter_context(tc.tile_pool(name="vh", bufs=2))
                e_pool = p2.enter_context(tc.tile_pool(name="eT", bufs=4))
                es_pool = p2.enter_context(tc.tile_pool(name="es", bufs=3))
                rs_pool = p2.enter_context(tc.tile_pool(name="rs", bufs=3))
                s_ps_pool = p2.enter_context(
                    tc.tile_pool(name="sps", bufs=3, space="PSUM"))
                o_ps_pool = p2.enter_context(
                    tc.tile_pool(name="ops", bufs=3, space="PSUM"))
                m_ps_pool = p2.enter_context(
                    tc.tile_pool(name="mps", bufs=2, space="PSUM"))

                pending = []

                def emit_epilogue():
                    if not pending:
                        return
                    h, qh, esum, o_ps = pending.pop(0)
                    m_ps = m_ps_pool.tile([128, 512], F32, tag="mps")
                    nc.tensor.matmul(m_ps[0:1, :], ones128[:], esum[:],
                                     start=True, stop=True)
                    rs = rs_pool.tile([1, 512], F32R, tag="rs")
                    with nc.allow_low_precision(reason="f32r is 4-byte"):
                        nc.vector.reciprocal(rs[:], m_ps[0:1, :])
                    nc.tensor.matmul(m_ps[:, :], ones1[:], rs[:],
                                     start=True, stop=True)
                    bc_sb = rs_pool.tile([128, 512], F32, tag="bc")
                    nc.scalar.copy(bc_sb[:], m_ps[:])
                    nc.vector.tensor_mul(
                        oT_all[:, h, ts(qh, 512)], o_ps[:], bc_sb[:])

                for h in range(H):
                    qh_sb = qh_pool.tile([128, NTQ], F32R, tag="qh")
                    nc.sync.dma_start(qh_sb[:], qTs[h * 128:(h + 1) * 128, :])
                    kh_sb = kh_pool.tile([128, NTKV], F32R, tag="kh")
                    nc.sync.dma_start(kh_sb[:], kTs[h * 128:(h + 1) * 128, :])
                    vh_sb = vh_pool.tile([128, KC * 128], F32R, tag="vh")
                    nc.sync.dma_start(vh_sb[:], vsc[h])
                    for qh in range(NTQ // 512):
                        qsl = qh_sb[:, ts(qh, 512)]
                        esum = es_pool.tile([128, 512], F32R, tag="es")
                        o_ps = o_ps_pool.tile([128, 512], F32, tag="ops")
                        eTs = [None] * KC
                        for kc in range(KC):
                            s_ps = s_ps_pool.tile([128, 512], F32, tag="sps")
                            nc.tensor.matmul(s_ps[:], kh_sb[:, ts(kc, 128)], qsl,
                                             start=True, stop=True)
                            eT = e_pool.tile([128, 512], F32R, tag="eT")
                            nc.scalar.activation(eT[:], s_ps[:], EXP, scale=SCALE)
                            eTs[kc] = eT
                            if kc == 0:
                                nc.vector.tensor_copy(esum[:], eT[:])
                            else:
                                nc.vector.tensor_add(esum[:], esum[:], eT[:])
                            if kc > 0:
                                nc.tensor.matmul(
                                    o_ps[:], vh_sb[:, ts(kc - 1, 128)], eTs[kc - 1][:],
                                    start=(kc == 1), stop=False)
                        nc.tensor.matmul(
                            o_ps[:], vh_sb[:, ts(KC - 1, 128)], eTs[KC - 1][:],
                            start=False, stop=True)
                        pending.append((h, qh, esum, o_ps))
                        if len(pending) > 1:
                            emit_epilogue()
                while pending:
                    emit_epilogue()

            # ----- phase 3: output projection -----
            with ExitStack() as p3:
                wp_pool = p3.enter_context(tc.tile_pool(name="wp", bufs=6))
                outev_pool = p3.enter_context(tc.tile_pool(name="outev", bufs=4))
                ps3 = p3.enter_context(tc.tile_pool(name="ps3", bufs=4, space="PSUM"))

                for ft in range(4):
                    wpl = []
                    for qt in range(4):
                        w_ = wp_pool.tile([128, 4, 512], F32R, tag="wp")
                        nc.sync.dma_start(w_[:], wps[ft, qt])
                        wpl.append(w_)
                    for tch in range(NTQ // 128):
                        ps = ps3.tile([128, 512], F32, tag="ps3")
                        for hc in range(H):
                            wp_ap = wpl[hc // 4][:, hc % 4, :]
                            nc.tensor.matmul(ps[:], oT_all[:, hc, ts(tch, 128)],
                                             wp_ap,
                                             start=(hc == 0), stop=(hc == H - 1))
                        oev = outev_pool.tile([128, 512], F32, tag="outev")
                        nc.scalar.copy(oev[:], ps[:])
                        nc.gpsimd.dma_start(
                            out[ts(tch, 128), ft * 512:(ft + 1) * 512], oev[:])

    _split_multi_waits(nc)
    return nc


# ---------------------------------------------------------------------------
# host-side prep / assembly
# ---------------------------------------------------------------------------

_ONES = np.ones((128, 128), dtype=np.float32)


def prep_inputs(x, w_attn, w_proj):
    x = np.asarray(x, dtype=np.float32)
    w_attn = np.asarray(w_attn, dtype=np.float32)
    w_proj = np.asarray(w_proj, dtype=np.float32)

    perm = np.concatenate([np.arange(0, HD, 2), np.arange(1, HD, 2)])
    colperm = (np.arange(H)[:, None] * HD + perm[None, :]).ravel()

    wq, wk, wv = w_attn[0:D], w_attn[D:2 * D], w_attn[2 * D:3 * D]
    # partition-major slabs: [h, p, cc*128] with wT[c, f] = w.T
    wqs = np.ascontiguousarray(
        wq.T[:, colperm].reshape(CC, 128, H, 128)
        .transpose(2, 1, 0, 3).reshape(H, 128, CC * 128))
    wks = np.ascontiguousarray(
        wk.T[:, colperm].reshape(CC, 128, H, 128)
        .transpose(2, 1, 0, 3).reshape(H, 128, CC * 128))
    # [ft, half, p, 8*512]
    wvs = np.ascontiguousarray(
        wv.T.reshape(4, 4, 128, 4, 512)
        .transpose(3, 0, 2, 1, 4).reshape(4, 4, 128, 4 * 512))
    wps = np.ascontiguousarray(
        w_proj.T.reshape(4, 4, 128, 4, 512)
        .transpose(3, 0, 2, 1, 4).reshape(4, 4, 128, 4 * 512))

    inv = 1.0 / (10000.0 ** (np.arange(0, HD, 2, dtype=np.float64) / HD))
    fr = np.outer(np.arange(T, dtype=np.float64), inv)
    cos = np.cos(fr).T
    sin = np.sin(fr).T
    cs2 = np.concatenate([cos, cos], 0).astype(np.float32)
    sn2 = np.concatenate([-sin, sin], 0).astype(np.float32)

    in_maps = []
    for i in range(N_CORES):
        b, half = i // 2, i % 2
        q0 = half * NTQ
        r0 = (1 - half) * NTQ
        xT_b = np.ascontiguousarray(x[b].T)
        # kv order: local half first, then remote half
        cs2k = np.concatenate([cs2[:, q0:q0 + NTQ], cs2[:, r0:r0 + NTQ]], axis=1)
        sn2k = np.concatenate([sn2[:, q0:q0 + NTQ], sn2[:, r0:r0 + NTQ]], axis=1)
        in_maps.append({
            "xTq": np.ascontiguousarray(xT_b[:, q0:q0 + NTQ]),
            "xTr": np.ascontiguousarray(xT_b[:, r0:r0 + NTQ]),
            "wqs": wqs, "wks": wks, "wvs": wvs, "wps": wps,
            "cs2q": np.ascontiguousarray(cs2[:, q0:q0 + NTQ]),
            "sn2q": np.ascontiguousarray(sn2[:, q0:q0 + NTQ]),
            "cs2k": np.ascontiguousarray(cs2k),
            "sn2k": np.ascontiguousarray(sn2k),
            "onesd": _ONES,
        })
    return in_maps


def assemble(results):
    out = np.empty((B, T, D), dtype=np.float32)
    for i in range(N_CORES):
        b, half = i // 2, i % 2
        out[b, half * NTQ:(half + 1) * NTQ, :] = results[i]["out"]
    return out


_nc_cache = None


def _get_nc():
    global _nc_cache
    if _nc_cache is None:
        _nc_cache = build_nc()
    return _nc_cache


def kernel(x, w_attn, w_proj):
    from concourse.bass_utils import run_bass_kernel_spmd
    nc = _get_nc()
    in_maps = prep_inputs(x, w_attn, w_proj)
    res = run_bass_kernel_spmd(nc, in_maps, list(range(N_CORES)))
    return assemble(res.results)


def run_profiled(x, w_attn, w_proj, trace_cores=None):
    """Like kernel() but with NTFF profiling; returns BassKernelResults."""
    from concourse.bass_utils import run_bass_kernel_spmd
    import sys as _sys, types as _types
    try:
        import antenv
        if "antenv.axon_hooks" not in _sys.modules:
            mod = _types.ModuleType("antenv.axon_hooks")
            _h = [None]
            mod.set_axon_ntff_profile_hook = lambda h: _h.__setitem__(0, h)
            mod.get_axon_ntff_profile_hook = lambda: _h[0]
            _sys.modules["antenv.axon_hooks"] = mod
            antenv.axon_hooks = mod
            from trn_agent_boot.trn_boot import _ntff_profile_via_ctypes
            mod.set_axon_ntff_profile_hook(
                _ntff_profile_via_ctypes('/opt/axon/libaxon_pjrt.so'))
    except Exception as e:  # profiling is best-effort
        print("profile hook setup failed:", e)
    nc = _get_nc()
    in_maps = prep_inputs(x, w_attn, w_proj)
    return run_bass_kernel_spmd(
        nc, in_maps, list(range(N_CORES)), trace=True,
        trace_cores=trace_cores if trace_cores is not None else [0])
